# revision 1
# baseline (speedup 1.0000x reference)
"""Distributed dot-product attention for TRN2, 8 NeuronCores.

Sharding: 8 cores = 4 batches x 2 head-groups (8 heads each).
Each core computes, for its (batch b, head-group g):
    Q = Xq[b] @ (Wq[g]/8).T ; K = Xk[b] @ Wk[g].T ; V = Xv[b] @ Wv[g].T
    per head h: A = exp(Q_h K_h^T); O_h = (A V_h) / rowsum(A)
    partial[b,g] = concat_h(O_h) @ Wc[:, g].T            (row-parallel)
Host: out[b] = partial[b,0] + partial[b,1] + bc          (all-reduce + bias)

Device-side layouts avoid every transpose: the host ships X^T and W^T in
bf16; S is computed transposed ([Lk, Lq]) so exp(S^T) feeds AV directly as
the moving operand; AV's stationary V carries a ones column so the softmax
denominators fall out of the same accumulation; composition consumes O^T
blocks as stationary operands and emits the natural-layout f32 partial.

Schedule: projections of head-pair p+1 and the V projection overlap the
exp-bound attention of pair p (Tile dataflow scheduling); the S^T matmuls
of a pair run concurrently on PE row-groups 0/64 via tile_position.
"""

import math
from contextlib import ExitStack

import numpy as np
import ml_dtypes

import concourse.bass as bass
import concourse.bacc as bacc
import concourse.tile as tile
from concourse import mybir
from concourse.bass_utils import run_bass_kernel_spmd

B, L, D, H = 4, 2048, 1024, 16
DH = D // H          # 64 per-head dim
HPC = H // 2         # 8 heads per core
G = HPC * DH         # 512 head-group width
N_CORES = 8

f32 = mybir.dt.float32
bf16 = mybir.dt.bfloat16


def build_nc(seq=L, debug=False):
    """Build the per-core Bass program (SPMD, identical on all cores)."""
    KD = D // 128        # 8 contraction chunks over model dim
    LK = seq // 128      # Lk chunks (16)
    NPAIR = HPC // 2     # 4 head-pairs
    NQ = seq // 512      # Lq quarters (4)
    MO = seq // 128      # output row chunks (16)

    nc = bacc.Bacc(None, target_bir_lowering=False, debug=False)

    # X^T inputs arrive tile-blocked: [(k, n), 128, 512] so every SBUF
    # load is one contiguous 128KB DMA (strided HWDGE reads mis-account
    # their completion semaphores and race their consumers).
    NQv = seq // 512
    xqT = nc.dram_tensor("xqT", [(D // 128) * NQv, 128, 512], bf16,
                         kind="ExternalInput")
    xkT = nc.dram_tensor("xkT", [D, seq], bf16, kind="ExternalInput")
    xvT = nc.dram_tensor("xvT", [(D // 128) * NQv, 128, 512], bf16,
                         kind="ExternalInput")
    wqT = nc.dram_tensor("wqT", [128, (D // 128) * G], bf16, kind="ExternalInput")
    wkT = nc.dram_tensor("wkT", [128, (D // 128) * G], bf16, kind="ExternalInput")
    wvT = nc.dram_tensor("wvT", [128, (D // 128) * G], bf16, kind="ExternalInput")
    wcT = nc.dram_tensor("wcT", [G, D], bf16, kind="ExternalInput")
    outp = nc.dram_tensor("outp", [seq, D], f32, kind="ExternalOutput")
    dbg = {}
    if debug:
        for nm, shp in [("qt", [4, 128, seq]), ("kt", [4, 128, seq]),
                        ("vt", [16, 128, 8, 65]), ("ot", [8, DH, seq]),
                        ("rcpb", [16, DH, 1024]), ("oc", [16, DH + 1, 1024])]:
            dbg[nm] = nc.dram_tensor(f"dbg_{nm}", shp, f32 if nm in ("rcpb", "oc") else bf16,
                                     kind="ExternalOutput")

    with tile.TileContext(nc) as tc, ExitStack() as ctx:
        Exp = mybir.ActivationFunctionType.Exp

        # Persistent SBUF: projected Q^T/K^T (pair tiles: head 2p on
        # partitions 0:64, head 2p+1 on 64:128), V with ones column,
        # normalized O^T, and the composition weight.
        const = ctx.enter_context(tc.tile_pool(name="const", bufs=1))
        QT_t = [const.tile([128, seq], bf16, tag=f"qt{p}", name=f"qt{p}")
                for p in range(NPAIR)]
        KT_t = [const.tile([128, seq], bf16, tag=f"kt{p}", name=f"kt{p}")
                for p in range(NPAIR)]
        V_t = [const.tile([128, HPC, DH + 1], bf16, tag=f"v{m}", name=f"v{m}")
               for m in range(LK)]
        OT_t = [const.tile([DH, seq], bf16, tag=f"ot{h}", name=f"ot{h}")
                for h in range(HPC)]
        wcT_t = [const.tile([DH, D], bf16, tag=f"wc{h}", name=f"wc{h}")
                 for h in range(HPC)]

        # Weights resident; X inputs streamed in 512-column rounds.
        wpool = ctx.enter_context(tc.tile_pool(name="wpool", bufs=1))

        def load_w(src, pfx, pool=None):
            w_all = (pool or wpool).tile([128, KD * G], bf16, tag=pfx,
                                         name=pfx, bufs=1)
            nc.gpsimd.dma_start(out=w_all[:], in_=src[:])
            return [w_all[:, k * G:(k + 1) * G] for k in range(KD)]

        wq_t = load_w(wqT, "wq")
        wk_t = load_w(wkT, "wk")

        xcol = ctx.enter_context(tc.tile_pool(name="xcol", bufs=2))

        def load_xcol(pfx, src, k, n, bufs=None, pool=None):
            t = (pool or xcol).tile([128, 512], bf16, tag=f"{pfx}{k}",
                                    name=f"{pfx}{k}_{n}", bufs=bufs)
            nc.gpsimd.dma_start(out=t[:], in_=src[k * NQ + n])
            return t

        # One shared PSUM pool for projections + composition (2 banks), the
        # S^T pair tiles (4 banks), and the AV accumulator (2 banks) = 8.
        gen_ps = ctx.enter_context(
            tc.tile_pool(name="gen_ps", bufs=2, space=bass.MemorySpace.PSUM))
        stp_p = ctx.enter_context(
            tc.tile_pool(name="stp", bufs=2, space=bass.MemorySpace.PSUM))
        oap = ctx.enter_context(
            tc.tile_pool(name="oap", bufs=1, space=bass.MemorySpace.PSUM))
        apool = ctx.enter_context(tc.tile_pool(name="apool", bufs=2))
        nrm = ctx.enter_context(tc.tile_pool(name="nrm", bufs=2))
        dscr = ctx.enter_context(
            tc.tile_pool(name="dscr", bufs=2, space=bass.MemorySpace.DRAM))

        def proj_qk_round(n, w_t, x_n, dst):
            """dst[p][:, n-cols] = (W block p).T @ X^T[:, n-cols], bf16."""
            for p in range(NPAIR):
                ps = gen_ps.tile([128, 512], f32, tag="pp", name="pp")
                for k in range(KD):
                    nc.tensor.matmul(
                        ps[:], lhsT=w_t[k][:, p * 128:(p + 1) * 128],
                        rhs=x_n[k][:], start=(k == 0), stop=(k == KD - 1))
                nc.vector.tensor_copy(dst[p][:, n * 512:(n + 1) * 512], ps[:])

        def proj_qk_pair(p, w_t, x_all, dst):
            """dst[p] over ALL column rounds (emitted before first use)."""
            for n in range(NQ):
                ps = gen_ps.tile([128, 512], f32, tag="pp", name="pp")
                for k in range(KD):
                    nc.tensor.matmul(
                        ps[:], lhsT=w_t[k][:, p * 128:(p + 1) * 128],
                        rhs=x_all[n][k][:], start=(k == 0), stop=(k == KD - 1))
                nc.vector.tensor_copy(dst[p][:, n * 512:(n + 1) * 512], ps[:])

        def proj_v_round(n, xv_n, wv_t):
            for mm in range(4):
                m = 4 * n + mm
                ps = gen_ps.tile([128, G], f32, tag="pp", name="pp")
                for k in range(KD):
                    nc.tensor.matmul(
                        ps[:], lhsT=xv_n[k][:, mm * 128:(mm + 1) * 128],
                        rhs=wv_t[k][:], start=(k == 0), stop=(k == KD - 1))
                nc.vector.tensor_copy(
                    V_t[m][:, :, 0:DH],
                    ps[:].rearrange("p (h d) -> p h d", h=HPC))
                nc.vector.memset(V_t[m][:, :, DH:DH + 1], 1.0)

        def attention_quarter(p, q):
                qsl = slice(q * 512, (q + 1) * 512)
                oacc = oap.tile([DH + 1, 1024], f32, tag="oacc", name="oacc")
                for lk in range(LK):
                    ksl = slice(lk * 128, (lk + 1) * 128)
                    stp = stp_p.tile([128, 1024], f32, tag="stp", name="stp")
                    # S^T for both heads, concurrent on PE row groups 0/64.
                    nc.tensor.matmul(
                        stp[:, 0:512], lhsT=KT_t[p][0:64, ksl],
                        rhs=QT_t[p][0:64, qsl], start=True, stop=True,
                        tile_position=(0, 0))
                    nc.tensor.matmul(
                        stp[:, 512:1024], lhsT=KT_t[p][64:128, ksl],
                        rhs=QT_t[p][64:128, qsl], start=True, stop=True,
                        tile_position=(64, 0))
                    a_sb = apool.tile([128, 1024], bf16, tag="a", name="a")
                    nc.scalar.activation(a_sb[:], stp[:], Exp)
                    # O^T (+denominator row DH) accumulated over Lk.
                    nc.tensor.matmul(
                        oacc[:, 0:512], lhsT=V_t[lk][:, 2 * p, :],
                        rhs=a_sb[:, 0:512],
                        start=(lk == 0), stop=(lk == LK - 1))
                    nc.tensor.matmul(
                        oacc[:, 512:1024], lhsT=V_t[lk][:, 2 * p + 1, :],
                        rhs=a_sb[:, 512:1024],
                        start=(lk == 0), stop=(lk == LK - 1))
                # Evict the accumulator so the next quarter can reuse PSUM.
                oc = nrm.tile([DH + 1, 1024], f32, tag="oc", name="oc")
                nc.vector.tensor_copy(oc[:], oacc[:])
                # Reciprocal of the denominators at full lane width: bounce
                # the row through DRAM reshaped to [64, 16].
                dn = dscr.tile([1, 1024], f32, tag="dn", name="dn")
                nc.gpsimd.dma_start(out=dn[:], in_=oc[DH:DH + 1, :])
                db = nrm.tile([DH, 16], f32, tag="db", name="db")
                nc.gpsimd.dma_start(
                    out=db[:], in_=dn[:].rearrange("o (p j) -> (o p) j", j=16))
                rb = nrm.tile([DH, 16], f32, tag="rb", name="rb")
                nc.vector.reciprocal(out=rb[:], in_=db[:])
                rd = dscr.tile([1, 1024], f32, tag="rd", name="rd")
                nc.gpsimd.dma_start(
                    out=rd[:].rearrange("o (p j) -> (o p) j", j=16), in_=rb[:])
                rcpb = nrm.tile([DH, 1024], f32, tag="rcpb", name="rcpb")
                nc.gpsimd.dma_start(out=rcpb[:], in_=rd[:].to_broadcast([DH, 1024]))
                if debug:
                    nc.sync.dma_start(out=dbg["rcpb"][4 * q + p], in_=rcpb[:])
                    nc.sync.dma_start(out=dbg["oc"][4 * q + p], in_=oc[:])
                # O^T rows per head, partitions 0:64 (no partition shift).
                nc.vector.tensor_mul(
                    OT_t[2 * p][:, qsl], oc[0:DH, 0:512], rcpb[:, 0:512])
                nc.vector.tensor_mul(
                    OT_t[2 * p + 1][:, qsl], oc[0:DH, 512:1024], rcpb[:, 512:1024])

        def comp_quarter(q):
            """Composition for output rows of quarter q (needs OT cols q)."""
            for m in range(4 * q, 4 * q + 4):
                msl = slice(m * 128, (m + 1) * 128)
                o_sb = ost.tile([128, D], f32, tag="osb", name="osb")
                for half in range(2):
                    hsl = slice(half * 512, (half + 1) * 512)
                    ps = gen_ps.tile([128, 512], f32, tag="pp", name="pp")
                    for h in range(HPC):
                        nc.tensor.matmul(
                            ps[:], lhsT=OT_t[h][:, msl], rhs=wcT_t[h][:, hsl],
                            start=(h == 0), stop=(h == HPC - 1))
                    nc.vector.tensor_copy(o_sb[:, hsl], ps[:])
                nc.gpsimd.dma_start(out=outp[msl, :], in_=o_sb[:])

        if debug:
            for p in range(NPAIR):
                nc.sync.dma_start(out=dbg["qt"][p], in_=QT_t[p][:])
                nc.sync.dma_start(out=dbg["kt"][p], in_=KT_t[p][:])
            for h in range(HPC):
                nc.sync.dma_start(out=dbg["ot"][h], in_=OT_t[h][:])
            for m in range(LK):
                nc.sync.dma_start(out=dbg["vt"][m], in_=V_t[m][:])

        # Program order = scheduling priority. Lead-in: half of V, then the
        # first Q^T/K^T column round, so the first exps fire ~30us in; the
        # rest of V/projections/composition fill PE slack under the
        # exp-bound attention quarters.
        # Emission order doubles as the dependency order: every tile a
        # consumer reads must already have its producer emitted, else Tile
        # records the access as write-after-read and never orders the read
        # behind the write. attention(p, q) reads ALL of K^T[p] and V but
        # only quarter q of Q^T - so V and K^T[p] are fully emitted before
        # their first consumer; Q^T streams one round ahead.
        with tc.tile_pool(name="xvw", bufs=2) as xvw:
            wv_t = load_w(wvT, "wv", pool=xvw)
            for n in range(NQ):
                xv_n = [load_xcol("xv", xvT, k, n, pool=xvw) for k in range(KD)]
                proj_v_round(n, xv_n, wv_t)
        xk_t = [xcol.tile([128, seq], bf16, tag=f"xk{k}", name=f"xk{k}",
                          bufs=1) for k in range(KD)]
        for k in range(KD):
            nc.gpsimd.dma_start(out=xk_t[k][:], in_=xkT[k * 128:(k + 1) * 128, :])
        xk_all = [[xk_t[k][:, n * 512:(n + 1) * 512] for k in range(KD)]
                  for n in range(NQ)]
        proj_qk_pair(0, wk_t, xk_all, KT_t)
        xq_n = [load_xcol("xq", xqT, k, 0) for k in range(KD)]
        proj_qk_round(0, wq_t, xq_n, QT_t)
        for h in range(HPC):
            nc.gpsimd.dma_start(out=wcT_t[h][:], in_=wcT[h * DH:(h + 1) * DH, :])
        ost = ctx.enter_context(tc.tile_pool(name="ost", bufs=3))

        for q in range(NQ):
            for p in range(NPAIR):
                if q == 0 and p + 1 < NPAIR:
                    proj_qk_pair(p + 1, wk_t, xk_all, KT_t)
                attention_quarter(p, q)
                if p == NPAIR - 2 and q + 1 < NQ:
                    # Next quarter's Q^T fills the last pair's exp slack
                    # (priority below it, above the next quarter).
                    xq_n = [load_xcol("xq", xqT, k, q + 1) for k in range(KD)]
                    proj_qk_round(q + 1, wq_t, xq_n, QT_t)
            comp_quarter(q)

    nc.compile()
    return nc


def shard_inputs(keys, queries, values, Wk, Wq, Wv, Wc, seq=L):
    """Host-side shard prep: per-core transposed bf16 operands."""

    def bf(a):
        return np.ascontiguousarray(a).astype(ml_dtypes.bfloat16)

    def bft(x):
        # [seq, D] -> X^T tile-blocked [(k, n), 128, 512]
        xt = np.ascontiguousarray(x.T).astype(ml_dtypes.bfloat16)
        kd, nq = xt.shape[0] // 128, xt.shape[1] // 512
        return np.ascontiguousarray(
            xt.reshape(kd, 128, nq, 512).transpose(0, 2, 1, 3)
        ).reshape(kd * nq, 128, 512)

    def wblk(w):
        # [D, G] -> [128, (D//128)*G] with k-block at cols k*G:(k+1)*G
        return bf(np.ascontiguousarray(
            np.asarray(w).reshape(D // 128, 128, G).transpose(1, 0, 2)
        ).reshape(128, (D // 128) * G))

    scale = 1.0 / math.sqrt(DH)
    in_maps = []
    for c in range(N_CORES):
        b, g = c // 2, c % 2
        gs = slice(g * G, (g + 1) * G)
        in_maps.append({
            "xqT": bft(queries[b, :seq]),
            "xkT": bf(keys[b, :seq].T),
            "xvT": bft(values[b, :seq]),
            "wqT": wblk(Wq[gs, :].T * scale),
            "wkT": wblk(Wk[gs, :].T),
            "wvT": wblk(Wv[gs, :].T),
            "wcT": bf(Wc[:, gs].T),
        })
    return in_maps


_NC_CACHE = {}


def run_cores(inputs, seq=L, trace=False):
    if seq not in _NC_CACHE:
        _NC_CACHE[seq] = build_nc(seq)
    nc = _NC_CACHE[seq]
    in_maps = shard_inputs(
        inputs["keys"], inputs["queries"], inputs["values"],
        inputs["Wk"], inputs["Wq"], inputs["Wv"], inputs["Wc"], seq=seq)
    res = run_bass_kernel_spmd(nc, in_maps, core_ids=list(range(N_CORES)),
                               trace=trace)
    return res


def kernel(keys, queries, values, Wk, Wq, Wv, Wc, bc, attn_mask):
    res = run_cores(dict(keys=np.asarray(keys), queries=np.asarray(queries),
                         values=np.asarray(values), Wk=np.asarray(Wk),
                         Wq=np.asarray(Wq), Wv=np.asarray(Wv),
                         Wc=np.asarray(Wc)))
    bc = np.asarray(bc, np.float32)
    out = np.empty((B, L, D), np.float32)
    for b in range(B):
        out[b] = res.results[2 * b]["outp"] + res.results[2 * b + 1]["outp"] + bc
    return out



# revision 4
# speedup vs baseline: 1.0722x; 1.0722x over previous
"""Distributed dot-product attention for TRN2, 8 NeuronCores.

Sharding: 8 cores = 4 batches x 2 head-groups (8 heads each).
Each core computes, for its (batch b, head-group g):
    Q = Xq[b] @ (Wq[g]/8).T ; K = Xk[b] @ Wk[g].T ; V = Xv[b] @ Wv[g].T
    per head h: A = exp(Q_h K_h^T); O_h = (A V_h) / rowsum(A)
    partial[b,g] = concat_h(O_h) @ Wc[:, g].T            (row-parallel)
Host: out[b] = partial[b,0] + partial[b,1] + bc          (all-reduce + bias)

Device-side layouts avoid every transpose: the host ships X^T and W^T in
bf16; S is computed transposed ([Lk, Lq]) so exp(S^T) feeds AV directly as
the moving operand; AV's stationary V carries a ones column so the softmax
denominators fall out of the same accumulation; composition consumes O^T
HEAD-PAIR blocks [128, m] (head 2p on partitions 0:64, head 2p+1 on
64:128 via a DVE quadrant-crossing write) so each comp matmul contracts
the full 128 rows; it emits the natural-layout f32 partial.

Schedule: the exp chain is the critical resource (ACT is the only engine
with the activation LUT; 33.5M exps/core ~ 280us busy). Lead-in projects
only K pair 0 + Q round 0 before the first S^T so exp starts ~25us in;
V/remaining-K/Q projections and the previous quarter's composition all
fill PE slack underneath the exp-bound attention quarters.
"""

import math
from contextlib import ExitStack

import numpy as np
import ml_dtypes

import concourse.bass as bass
import concourse.bacc as bacc
import concourse.tile as tile
from concourse import mybir
from concourse.bass_utils import run_bass_kernel_spmd

B, L, D, H = 4, 2048, 1024, 16
DH = D // H          # 64 per-head dim
HPC = H // 2         # 8 heads per core
G = HPC * DH         # 512 head-group width
N_CORES = 8

f32 = mybir.dt.float32
bf16 = mybir.dt.bfloat16


def build_nc(seq=L, debug=False):
    """Build the per-core Bass program (SPMD, identical on all cores)."""
    KD = D // 128        # 8 contraction chunks over model dim
    LK = seq // 128      # Lk chunks (16)
    NPAIR = HPC // 2     # 4 head-pairs
    NQ = seq // 512      # Lq quarters (4)

    nc = bacc.Bacc(None, target_bir_lowering=False, debug=False)

    # X^T inputs arrive tile-blocked: [(k, n), 128, 512] so every SBUF
    # load is one contiguous 128KB DMA (strided HWDGE reads mis-account
    # their completion semaphores and race their consumers).
    NQv = seq // 512
    xqT = nc.dram_tensor("xqT", [(D // 128) * NQv, 128, 512], bf16,
                         kind="ExternalInput")
    xkT = nc.dram_tensor("xkT", [(D // 128) * NQv, 128, 512], bf16,
                         kind="ExternalInput")
    xvT = nc.dram_tensor("xvT", [(D // 128) * NQv, 128, 512], bf16,
                         kind="ExternalInput")
    wqT = nc.dram_tensor("wqT", [128, (D // 128) * G], bf16, kind="ExternalInput")
    wkT = nc.dram_tensor("wkT", [128, (D // 128) * G], bf16, kind="ExternalInput")
    wvT = nc.dram_tensor("wvT", [128, (D // 128) * G], bf16, kind="ExternalInput")
    wcT = nc.dram_tensor("wcT", [G, D], bf16, kind="ExternalInput")
    outp = nc.dram_tensor("outp", [seq, D], f32, kind="ExternalOutput")
    dbg = {}
    if debug:
        for nm, shp in [("qt", [4, 128, seq]), ("kt", [4, 128, seq]),
                        ("vt", [16, 128, 8, 65]), ("ot", [4, 128, seq]),
                        ("rcpb", [16, DH, 1024]), ("oc", [16, DH + 1, 1024])]:
            dbg[nm] = nc.dram_tensor(f"dbg_{nm}", shp, f32 if nm in ("rcpb", "oc") else bf16,
                                     kind="ExternalOutput")

    with tile.TileContext(nc) as tc, ExitStack() as ctx:
        Exp = mybir.ActivationFunctionType.Exp

        # Persistent SBUF: projected Q^T/K^T (pair tiles: head 2p on
        # partitions 0:64, head 2p+1 on 64:128), V with ones column,
        # normalized O^T as head-pair tiles, and the composition weight
        # as head-pair tiles [128, D].
        const = ctx.enter_context(tc.tile_pool(name="const", bufs=1))
        QT_t = [const.tile([128, seq], bf16, tag=f"qt{p}", name=f"qt{p}")
                for p in range(NPAIR)]
        KT_t = [const.tile([128, seq], bf16, tag=f"kt{p}", name=f"kt{p}")
                for p in range(NPAIR)]
        V_t = [const.tile([128, HPC, DH + 1], bf16, tag=f"v{m}", name=f"v{m}")
               for m in range(LK)]
        OTP_t = [const.tile([128, seq], bf16, tag=f"otp{p}", name=f"otp{p}")
                 for p in range(NPAIR)]
        wcP_t = [const.tile([128, D], bf16, tag=f"wcp{p}", name=f"wcp{p}")
                 for p in range(NPAIR)]

        # Weights resident; X inputs streamed in 512-column rounds.
        wpool = ctx.enter_context(tc.tile_pool(name="wpool", bufs=1))

        def load_w(src, pfx, pool=None):
            w_all = (pool or wpool).tile([128, KD * G], bf16, tag=pfx,
                                         name=pfx, bufs=1)
            nc.gpsimd.dma_start(out=w_all[:], in_=src[:])
            return [w_all[:, k * G:(k + 1) * G] for k in range(KD)]

        xcol = ctx.enter_context(tc.tile_pool(name="xcol", bufs=2))

        def load_xcol(pfx, src, k, n, bufs=None, pool=None):
            t = (pool or xcol).tile([128, 512], bf16, tag=f"{pfx}{k}",
                                    name=f"{pfx}{k}_{n}", bufs=bufs)
            nc.gpsimd.dma_start(out=t[:], in_=src[k * NQ + n])
            return t

        # One shared PSUM pool for projections + composition (2 banks), the
        # S^T pair tiles (4 banks), and the AV accumulator (2 banks) = 8.
        gen_ps = ctx.enter_context(
            tc.tile_pool(name="gen_ps", bufs=2, space=bass.MemorySpace.PSUM))
        stp_p = ctx.enter_context(
            tc.tile_pool(name="stp", bufs=2, space=bass.MemorySpace.PSUM))
        oap = ctx.enter_context(
            tc.tile_pool(name="oap", bufs=1, space=bass.MemorySpace.PSUM))
        apool = ctx.enter_context(tc.tile_pool(name="apool", bufs=2))
        nrm = ctx.enter_context(tc.tile_pool(name="nrm", bufs=2))
        dscr = ctx.enter_context(
            tc.tile_pool(name="dscr", bufs=2, space=bass.MemorySpace.DRAM))

        def proj_qk_round(n, w_t, x_n, dst, pairs=None):
            """dst[p][:, n-cols] = (W block p).T @ X^T[:, n-cols], bf16."""
            for p in (pairs if pairs is not None else range(NPAIR)):
                ps = gen_ps.tile([128, 512], f32, tag="pp", name="pp")
                for k in range(KD):
                    nc.tensor.matmul(
                        ps[:], lhsT=w_t[k][:, p * 128:(p + 1) * 128],
                        rhs=x_n[k][:], start=(k == 0), stop=(k == KD - 1))
                nc.vector.tensor_copy(dst[p][:, n * 512:(n + 1) * 512], ps[:])

        def proj_qk_pair(p, w_t, x_all, dst):
            """dst[p] over ALL column rounds (emitted before first use)."""
            for n in range(NQ):
                ps = gen_ps.tile([128, 512], f32, tag="pp", name="pp")
                for k in range(KD):
                    nc.tensor.matmul(
                        ps[:], lhsT=w_t[k][:, p * 128:(p + 1) * 128],
                        rhs=x_all[n][k][:], start=(k == 0), stop=(k == KD - 1))
                nc.vector.tensor_copy(dst[p][:, n * 512:(n + 1) * 512], ps[:])

        def proj_v_round(n, xv_n, wv_t):
            for mm in range(4):
                m = 4 * n + mm
                ps = gen_ps.tile([128, G], f32, tag="pp", name="pp")
                for k in range(KD):
                    nc.tensor.matmul(
                        ps[:], lhsT=xv_n[k][:, mm * 128:(mm + 1) * 128],
                        rhs=wv_t[k][:], start=(k == 0), stop=(k == KD - 1))
                nc.vector.tensor_copy(
                    V_t[m][:, :, 0:DH],
                    ps[:].rearrange("p (h d) -> p h d", h=HPC))
                nc.vector.memset(V_t[m][:, :, DH:DH + 1], 1.0)

        def attention_quarter(p, q):
                qsl = slice(q * 512, (q + 1) * 512)
                oacc = oap.tile([DH + 1, 1024], f32, tag="oacc", name="oacc")
                for lk in range(LK):
                    ksl = slice(lk * 128, (lk + 1) * 128)
                    stp = stp_p.tile([128, 1024], f32, tag="stp", name="stp")
                    # S^T for both heads, concurrent on PE row groups 0/64.
                    nc.tensor.matmul(
                        stp[:, 0:512], lhsT=KT_t[p][0:64, ksl],
                        rhs=QT_t[p][0:64, qsl], start=True, stop=True,
                        tile_position=(0, 0))
                    nc.tensor.matmul(
                        stp[:, 512:1024], lhsT=KT_t[p][64:128, ksl],
                        rhs=QT_t[p][64:128, qsl], start=True, stop=True,
                        tile_position=(64, 0))
                    a_sb = apool.tile([128, 1024], bf16, tag="a", name="a")
                    nc.scalar.activation(a_sb[:], stp[:], Exp)
                    # O^T (+denominator row DH) accumulated over Lk.
                    nc.tensor.matmul(
                        oacc[:, 0:512], lhsT=V_t[lk][:, 2 * p, :],
                        rhs=a_sb[:, 0:512],
                        start=(lk == 0), stop=(lk == LK - 1))
                    nc.tensor.matmul(
                        oacc[:, 512:1024], lhsT=V_t[lk][:, 2 * p + 1, :],
                        rhs=a_sb[:, 512:1024],
                        start=(lk == 0), stop=(lk == LK - 1))
                # Evict the accumulator so the next quarter can reuse PSUM.
                oc = nrm.tile([DH + 1, 1024], f32, tag="oc", name="oc")
                nc.vector.tensor_copy(oc[:], oacc[:])
                # Reciprocal of the denominators at full lane width via a
                # DRAM bounce reshaped to [64, 16].
                dn = dscr.tile([1, 1024], f32, tag="dn", name="dn")
                nc.sync.dma_start(out=dn[:], in_=oc[DH:DH + 1, :])
                db = nrm.tile([DH, 16], f32, tag="db", name="db")
                nc.sync.dma_start(
                    out=db[:], in_=dn[:].rearrange("o (p j) -> (o p) j", j=16))
                rb = nrm.tile([DH, 16], f32, tag="rb", name="rb")
                nc.vector.reciprocal(out=rb[:], in_=db[:])
                rd = dscr.tile([1, 1024], f32, tag="rd", name="rd")
                nc.sync.dma_start(
                    out=rd[:].rearrange("o (p j) -> (o p) j", j=16), in_=rb[:])
                rcpb = nrm.tile([DH, 1024], f32, tag="rcpb", name="rcpb")
                nc.sync.dma_start(out=rcpb[:], in_=rd[:].to_broadcast([DH, 1024]))
                if debug:
                    nc.sync.dma_start(out=dbg["rcpb"][4 * q + p], in_=rcpb[:])
                    nc.sync.dma_start(out=dbg["oc"][4 * q + p], in_=oc[:])
                # Normalized O^T into the head-pair tile: head 2p on
                # partitions 0:64, head 2p+1 on 64:128 (DVE 64-channel op
                # writing the upper quadrants).
                nc.vector.tensor_mul(
                    OTP_t[p][0:DH, qsl], oc[0:DH, 0:512], rcpb[:, 0:512])
                nc.vector.tensor_mul(
                    OTP_t[p][DH:128, qsl], oc[0:DH, 512:1024], rcpb[:, 512:1024])

        def comp_quarter(q):
            """Composition for output rows of quarter q (needs OTP cols q)."""
            for m in range(4 * q, 4 * q + 4):
                msl = slice(m * 128, (m + 1) * 128)
                o_sb = ost.tile([128, D], f32, tag="osb", name="osb")
                for half in range(2):
                    hsl = slice(half * 512, (half + 1) * 512)
                    ps = gen_ps.tile([128, 512], f32, tag="pp", name="pp")
                    for p in range(NPAIR):
                        nc.tensor.matmul(
                            ps[:], lhsT=OTP_t[p][:, msl], rhs=wcP_t[p][:, hsl],
                            start=(p == 0), stop=(p == NPAIR - 1))
                    nc.vector.tensor_copy(o_sb[:, hsl], ps[:])
                nc.sync.dma_start(out=outp[msl, :], in_=o_sb[:])

        if debug:
            for p in range(NPAIR):
                nc.sync.dma_start(out=dbg["qt"][p], in_=QT_t[p][:])
                nc.sync.dma_start(out=dbg["kt"][p], in_=KT_t[p][:])
                nc.sync.dma_start(out=dbg["ot"][p], in_=OTP_t[p][:])
            for m in range(LK):
                nc.sync.dma_start(out=dbg["vt"][m], in_=V_t[m][:])

        # Program order = scheduling priority AND dependency order: every
        # tile a consumer reads must already have its producer emitted,
        # else Tile records the access as write-after-read and never
        # orders the read behind the write.
        #
        # Lead-in: ONLY K pair 0 + Q round 0 gate the first S^T, so the
        # exp chain (the critical resource) starts as soon as ~6MB of
        # DMA + ~13us of PE have run. V rounds are emitted next (AV of
        # chunk lk waits on V[lk] at runtime, and the first AVs trail the
        # first exps by ~1us); remaining K pairs / Q rounds / previous
        # quarter's composition fill PE slack under the exp-bound
        # attention quarters.
        wk_t = load_w(wkT, "wk")
        xk_t = [[load_xcol(f"xk{n}_", xkT, k, n, bufs=1) for k in range(KD)]
                for n in range(NQ)]
        proj_qk_pair(0, wk_t, xk_t, KT_t)
        wq_t = load_w(wqT, "wq")
        xq_n = [load_xcol("xq", xqT, k, 0) for k in range(KD)]
        proj_qk_round(0, wq_t, xq_n, QT_t)

        with tc.tile_pool(name="xvw", bufs=2) as xvw:
            wv_t = load_w(wvT, "wv", pool=xvw)
            for n in range(NQ):
                xv_n = [load_xcol("xv", xvT, k, n, pool=xvw) for k in range(KD)]
                proj_v_round(n, xv_n, wv_t)
        for p in range(NPAIR):
            nc.gpsimd.dma_start(out=wcP_t[p][:], in_=wcT[p * 128:(p + 1) * 128, :])
        ost = ctx.enter_context(tc.tile_pool(name="ost", bufs=3))

        for q in range(NQ):
            for p in range(NPAIR):
                if q == 0 and p + 1 < NPAIR:
                    proj_qk_pair(p + 1, wk_t, xk_t, KT_t)
                attention_quarter(p, q)
                if p == 0 and q > 0:
                    # Previous quarter's composition fills PE slack under
                    # this quarter's exp-bound attention (instead of
                    # stalling the exp chain at the quarter boundary).
                    comp_quarter(q - 1)
                if p == NPAIR - 2 and q + 1 < NQ:
                    # Next quarter's Q^T fills the last pair's exp slack
                    # (priority below it, above the next quarter).
                    xq_n = [load_xcol("xq", xqT, k, q + 1) for k in range(KD)]
                    proj_qk_round(q + 1, wq_t, xq_n, QT_t)
            if q == NQ - 1:
                comp_quarter(q)

    nc.compile()
    return nc


def shard_inputs(keys, queries, values, Wk, Wq, Wv, Wc, seq=L):
    """Host-side shard prep: per-core transposed bf16 operands."""

    def bf(a):
        return np.ascontiguousarray(a).astype(ml_dtypes.bfloat16)

    def bft(x):
        # [seq, D] -> X^T tile-blocked [(k, n), 128, 512]
        xt = np.ascontiguousarray(x.T).astype(ml_dtypes.bfloat16)
        kd, nq = xt.shape[0] // 128, xt.shape[1] // 512
        return np.ascontiguousarray(
            xt.reshape(kd, 128, nq, 512).transpose(0, 2, 1, 3)
        ).reshape(kd * nq, 128, 512)

    def wblk(w):
        # [D, G] -> [128, (D//128)*G] with k-block at cols k*G:(k+1)*G
        return bf(np.ascontiguousarray(
            np.asarray(w).reshape(D // 128, 128, G).transpose(1, 0, 2)
        ).reshape(128, (D // 128) * G))

    scale = 1.0 / math.sqrt(DH)
    in_maps = []
    for c in range(N_CORES):
        b, g = c // 2, c % 2
        gs = slice(g * G, (g + 1) * G)
        in_maps.append({
            "xqT": bft(queries[b, :seq]),
            "xkT": bft(keys[b, :seq]),
            "xvT": bft(values[b, :seq]),
            "wqT": wblk(Wq[gs, :].T * scale),
            "wkT": wblk(Wk[gs, :].T),
            "wvT": wblk(Wv[gs, :].T),
            "wcT": bf(Wc[:, gs].T),
        })
    return in_maps


_NC_CACHE = {}


def run_cores(inputs, seq=L, trace=False):
    if seq not in _NC_CACHE:
        _NC_CACHE[seq] = build_nc(seq)
    nc = _NC_CACHE[seq]
    in_maps = shard_inputs(
        inputs["keys"], inputs["queries"], inputs["values"],
        inputs["Wk"], inputs["Wq"], inputs["Wv"], inputs["Wc"], seq=seq)
    res = run_bass_kernel_spmd(nc, in_maps, core_ids=list(range(N_CORES)),
                               trace=trace)
    return res


def kernel(keys, queries, values, Wk, Wq, Wv, Wc, bc, attn_mask):
    res = run_cores(dict(keys=np.asarray(keys), queries=np.asarray(queries),
                         values=np.asarray(values), Wk=np.asarray(Wk),
                         Wq=np.asarray(Wq), Wv=np.asarray(Wv),
                         Wc=np.asarray(Wc)))
    bc = np.asarray(bc, np.float32)
    out = np.empty((B, L, D), np.float32)
    for b in range(B):
        out[b] = res.results[2 * b]["outp"] + res.results[2 * b + 1]["outp"] + bc
    return out
